# revision 11
# baseline (speedup 1.0000x reference)
"""Squared Euclidean distance matrix kernel for Trainium2 (Bass/Tile).

out[i, j] = ||mat_1[i]||^2 + ||mat_2[j]||^2 - 2 * mat_1[i] . mat_2[j]

Sharding: mat_1 rows (and hence output rows) split across 8 NeuronCores;
mat_2 replicated.  Each core computes a (2048, 8192) tile independently.

Per-core plan (v2 — corrections folded into the GEMM, engines balanced):
  - Bt[k] = -2 * B^T chunk k  (128 x 8192 f32r), PE transposes + DVE scale.
  - At[k] =      A^T chunk k  (128 x 2048 f32r), PE transposes + DVE copy.
  - Row vectors on partition 0/1 for a rank-2 correction matmul:
      corr_lhsT = [ones_row; sqa_row]   (2 x 2048)
      corr_rhs  = [sqb_row;  ones_row]  (2 x 8192)
    so  corr_lhsT^T @ corr_rhs = sq_a[m] + sq_b[n].
    sq_b = 0.25 * sum((-2B)^2) via an 0.25-column matmul over Bt^2;
    sq_a via a ones-column matmul over At^2; ones via (1/128)-column
    matmul over an all-ones tile.  Partition-1 rows are placed with
    one-time SBUF->SBUF DMAs.
  - main: psum = At^T@Bt (+ correction matmul)  ->  plain copy to the
    staging buffer, alternating DVE / ACT  ->  2MB contiguous DMA out.
"""

import numpy as np

import concourse.bass as bass
import concourse.mybir as mybir
from concourse import bacc
from contextlib import ExitStack
from concourse.tile import TileContext
from concourse.masks import make_identity

F32 = mybir.dt.float32
F32R = mybir.dt.float32r
AX = mybir.AxisListType
OP = mybir.AluOpType
AF = mybir.ActivationFunctionType

N_CORES = 8
M_FULL, N_FULL, D_FULL = 16384, 8192, 256


def build(m_sh=M_FULL // N_CORES, n=N_FULL, d=D_FULL):
    P = 128
    FD = 512                      # matmul moving free dim (1 PSUM bank fp32)
    KC = d // P                   # contraction chunks
    MT = m_sh // P                # m tiles per core
    NS = n // FD                  # n slices
    NB = n // P                   # b row tiles
    AB = min(4, m_sh // P)        # a row tiles per load batch
    out_w = min(4096, n)          # out staging width
    OH = n // out_w
    SPW = out_w // FD             # slices per staging buffer
    FDm = min(FD, m_sh)           # row-vector chunk width along m
    MS = m_sh // FDm              # FDm-wide slices of the m axis

    nc = bacc.Bacc()
    a = nc.dram_tensor("a", [m_sh, d], F32, kind="ExternalInput")
    b = nc.dram_tensor("b", [n, d], F32, kind="ExternalInput")
    o = nc.dram_tensor("out", [m_sh, n], F32, kind="ExternalOutput")

    with ExitStack() as ctx:
        tc = ctx.enter_context(TileContext(nc))
        singles = ctx.enter_context(tc.tile_pool(name="singles", bufs=1))
        persist = ctx.enter_context(tc.tile_pool(name="persist", bufs=1))
        natp = ctx.enter_context(tc.tile_pool(name="natp", bufs=3))
        tmpp = ctx.enter_context(tc.tile_pool(name="tmpp", bufs=3))
        outp = ctx.enter_context(tc.tile_pool(name="outp", bufs=2))
        tpp = ctx.enter_context(tc.tile_pool(name="tpp", bufs=2, space="PSUM"))
        rowp = ctx.enter_context(tc.tile_pool(name="rowp", bufs=2, space="PSUM"))
        mmp = ctx.enter_context(tc.tile_pool(name="mmp", bufs=4, space="PSUM"))

        identity = singles.tile([P, P], F32, tag="identity", name="identity")
        make_identity(nc, identity)

        # f32r constants (memset f32 staging, rounded copy into f32r)
        ones_f = singles.tile([P, FD], F32, tag="ones_f", name="ones_f")
        nc.vector.memset(ones_f, 1.0)
        allones = singles.tile([P, FD], F32R, tag="allones", name="allones")
        nc.vector.tensor_copy(allones, ones_f)
        cst_f = singles.tile([P, 2], F32, tag="cst_f", name="cst_f")
        nc.vector.memset(cst_f[:, 0:1], 0.25)
        nc.vector.memset(cst_f[:, 1:2], 1.0 / 128.0)
        cst = singles.tile([P, 2], F32R, tag="cst", name="cst")
        nc.vector.tensor_copy(cst, cst_f)
        qcol = cst[:, 0:1]        # 0.25 column
        rcol = cst[:, 1:2]        # 1/128 column
        onecol = allones[:, 0:1]  # 1.0 column

        bts = [
            persist.tile([P, n], F32R, tag=f"bt{k}", name=f"bt{k}")
            for k in range(KC)
        ]
        ats = [
            persist.tile([P, m_sh], F32R, tag=f"at{k}", name=f"at{k}")
            for k in range(KC)
        ]
        corr_lhsT = persist.tile([2, m_sh], F32R, tag="corr_l", name="corr_lhsT")
        corr_rhs = persist.tile([2, n], F32R, tag="corr_r", name="corr_rhs")
        sqa_row = persist.tile([1, m_sh], F32R, tag="sqa_row", name="sqa_row")

        # ---- ones rows: ps_ones = (1/128 col)^T @ allones = row of 1.0 ----
        ps_ones = rowp.tile([1, FDm], F32, tag="row", name="ps_ones")
        nc.tensor.matmul(ps_ones, rcol, allones[:, :FDm], start=True, stop=True)
        for i in range(m_sh // FDm):
            nc.vector.tensor_copy(corr_lhsT[0:1, i * FDm:(i + 1) * FDm], ps_ones)

        # ---- Phase 0a: load B, build Bt = -2*B^T ----
        for t in range(NB):
            b_nat = natp.tile([P, d], F32, tag="nat", name="b_nat")
            nc.sync.dma_start(out=b_nat, in_=b[t * P:(t + 1) * P, :])
            for k in range(KC):
                pt = tpp.tile([P, P], F32, tag="tp", name="pt")
                nc.tensor.transpose(pt, b_nat[:, k * P:(k + 1) * P], identity)
                nc.vector.tensor_scalar_mul(bts[k][:, t * P:(t + 1) * P], pt, -2.0)

        # ---- Phase 0b: load A, build At = A^T ----
        for t4 in range(MT // AB):
            a_nat = natp.tile([P, AB, d], F32, tag="anat", name="a_nat")
            src = a[t4 * AB * P:(t4 + 1) * AB * P, :].rearrange(
                "(t p) d -> p t d", p=P
            )
            nc.sync.dma_start(out=a_nat, in_=src)
            for j in range(AB):
                mt = t4 * AB + j
                for k in range(KC):
                    pt = tpp.tile([P, P], F32, tag="tp", name="pt_a")
                    nc.tensor.transpose(pt, a_nat[:, j, k * P:(k + 1) * P], identity)
                    nc.vector.tensor_copy(ats[k][:, mt * P:(mt + 1) * P], pt)

        # ---- Phase 0c: sqb row = 0.25 * colsum(Bt^2), into corr_rhs[0] ----
        for s in range(NS):
            nsl = slice(s * FD, (s + 1) * FD)
            ps = rowp.tile([1, FD], F32, tag="row", name="ps_sqb")
            for k in range(KC):
                bsq = tmpp.tile([P, FD], F32R, tag="bsq", name="bsq")
                nc.vector.tensor_mul(bsq, bts[k][:, nsl], bts[k][:, nsl])
                nc.tensor.matmul(ps, qcol, bsq, start=(k == 0), stop=(k == KC - 1))
            nc.vector.tensor_copy(corr_rhs[0:1, nsl], ps)

        # ---- Phase 0d: sqa row = colsum(At^2), into sqa_row ----
        for s in range(MS):
            msl = slice(s * FDm, (s + 1) * FDm)
            ps = rowp.tile([1, FDm], F32, tag="row", name="ps_sqa")
            for k in range(KC):
                asq = tmpp.tile([P, FDm], F32R, tag="bsq", name="asq")
                nc.vector.tensor_mul(asq, ats[k][:, msl], ats[k][:, msl])
                nc.tensor.matmul(ps, onecol, asq, start=(k == 0), stop=(k == KC - 1))
            nc.vector.tensor_copy(sqa_row[0:1, msl], ps)

        # ---- Phase 0e: partition-1 rows via SBUF->SBUF DMA ----
        nc.sync.dma_start(out=corr_lhsT[1:2, :], in_=sqa_row[0:1, :])
        for i in range(n // m_sh):
            nc.sync.dma_start(
                out=corr_rhs[1:2, i * m_sh:(i + 1) * m_sh],
                in_=corr_lhsT[0:1, :],
            )

        # ---- Phase 1: main loop ----
        for m in range(MT):
            msl = slice(m * P, (m + 1) * P)
            for h in range(OH):
                ostage = outp.tile([P, out_w], F32, tag="ostage", name="ostage")
                for sj in range(SPW):
                    s = h * SPW + sj
                    nsl = slice(s * FD, (s + 1) * FD)
                    ps = mmp.tile([P, FD], F32, tag="mm", name="ps_mm")
                    for k in range(KC):
                        nc.tensor.matmul(
                            ps, ats[k][:, msl], bts[k][:, nsl],
                            start=(k == 0), stop=False,
                        )
                    nc.tensor.matmul(
                        ps, corr_lhsT[:, msl], corr_rhs[:, nsl],
                        start=False, stop=True,
                    )
                    osl = ostage[:, sj * FD:(sj + 1) * FD]
                    if (m + h + sj) % 2 == 0:
                        nc.vector.tensor_copy(osl, ps)
                    else:
                        nc.scalar.activation(osl, ps, AF.Copy)
                nc.sync.dma_start(
                    out=o[msl, h * out_w:(h + 1) * out_w], in_=ostage
                )
    nc.finalize()
    return nc


_CACHE = {}


def _get_nc():
    if "nc" not in _CACHE:
        _CACHE["nc"] = build()
    return _CACHE["nc"]


def run(mat_1, mat_2, trace=False):
    from concourse.bass_utils import run_bass_kernel_spmd

    a = np.ascontiguousarray(np.asarray(mat_1, dtype=np.float32))
    b = np.ascontiguousarray(np.asarray(mat_2, dtype=np.float32))
    assert a.shape == (M_FULL, D_FULL) and b.shape == (N_FULL, D_FULL)
    m_sh = M_FULL // N_CORES
    nc = _get_nc()
    in_maps = [
        {"a": a[c * m_sh:(c + 1) * m_sh], "b": b} for c in range(N_CORES)
    ]
    res = run_bass_kernel_spmd(nc, in_maps, core_ids=list(range(N_CORES)), trace=trace)
    out = np.concatenate([r["out"] for r in res.results], axis=0)
    return out, res


def kernel(mat_1, mat_2):
    return run(mat_1, mat_2)[0]
